# revision 17
# baseline (speedup 1.0000x reference)
"""Trainium2 Bass kernel for nn_Encoder_61177514164477 (meta-GCN LSTM encoder).

Sharding: 8 cores = 4 batch groups x 2 node-halves. Core c handles batch
b = c//2 and node rows [half*1024, (half+1)*1024) with half = c%2.
G^T (fp8 e4m3, x512) stays SBUF-resident per core; one pairwise masked
ReduceScatter per timestep exchanges h0/h1 (fp8, x16) between the two
halves of each batch pair.

Layer-0's einsum1 (G @ [x_t | h0]) is eliminated: G@h0_t already falls out
of layer-1's einsum1 (rows 0:64 of supT1 = G @ [h0_t | h1_{t-1}]), so
layer-0's conv at step t+1 reuses supT1 rows 0:64 as its stationary. The
tiny known-ahead G@x_t part is computed on the host and shipped as a
49-row bf16 stationary (48 GX rows for all (k,t,c) plus a ones row that
carries the conv bias through the weight matrix).

einsum1 runs in fp8 DoubleRow mode (2 j-tiles per MM, 2x PE throughput):
G scaled x512 and h x16 to stay in e4m3 range; the 8192x factor is divided
out of the host-computed W0h/W1. Offline sim: end-to-end rel err ~1.0e-2.

Per step the PE does:
  einsum1: 8 jp x 6 (k,ih) DoubleRow MMs @N=512 (single-phase PSUM accum)
  conv0:   8 it x (3 h-MMs contraction 64 + 1 x/bias-MM contraction 49) @N=256
  conv1:   8 it x 3 k MMs @N=256   (bf16)
"""
import os
import numpy as np
import ml_dtypes

import concourse.bass as bass
import concourse.mybir as mybir
import concourse.tile as tile
import concourse.bacc as bacc
import concourse.tile_utils as tile_utils
from concourse.bass_utils import run_bass_kernel_spmd

# use the full cayman SBUF (224 KiB phys / ~208 usable per partition)
tile_utils.max_sbuf_usage = 204 * 1024

L, B, T, N, C, H, K, M = 2, 4, 8, 2048, 2, 64, 3, 32
DIN0, DIN1, DOUT = C + H, 2 * H, 4 * H
HALF = N // 2          # 1024 rows per core
JT = N // 128          # 16 j-tiles (local order: 8 own + 8 partner)
JP = JT // 2           # 8 j-tile pairs (DoubleRow: 4 own + 4 partner)
IT = HALF // 128       # 8 own i-tiles
NCORES = 8
PAIRS = [[0, 1], [2, 3], [4, 5], [6, 7]]

F32 = mybir.dt.float32
BF16 = mybir.dt.bfloat16
F8 = mybir.dt.float8e4
DR = mybir.MatmulPerfMode.DoubleRow
BF = ml_dtypes.bfloat16
E4 = ml_dtypes.float8_e4m3fn

SC1 = DIN1             # 128 stationary cols per j-tile ([h0|h1])
XR = K * T * C + 1     # 49 rows of the x/bias stationary (48 GX + ones)
GS = 512.0             # G fp8 scale
HS = 16.0              # h fp8 scale

_CACHE = {}
LAST_RESULT = None


def _build():
    if "nc" in _CACHE:
        return _CACHE["nc"]
    nc = bacc.Bacc(None, target_bir_lowering=False, debug=False)

    gt_in = nc.declare_dram_parameter("gt2", [JP, 128, 2 * K * HALF], F8,
                                      isOutput=False)
    gxt_in = nc.declare_dram_parameter("gxt", [XR, HALF], BF16, isOutput=False)
    w0h_in = nc.declare_dram_parameter("w0h", [H, T * K * DOUT], BF16, isOutput=False)
    w0xb_in = nc.declare_dram_parameter("w0xb", [XR, T * DOUT], BF16, isOutput=False)
    w1_in = nc.declare_dram_parameter("w1", [SC1, T * K * DOUT], BF16, isOutput=False)
    b1_in = nc.declare_dram_parameter("bias1", [128, T * DOUT], F32, isOutput=False)
    supi_in = nc.declare_dram_parameter("sup_init", [H, K * HALF], BF16, isOutput=False)
    h1i_in = nc.declare_dram_parameter("h1_init", [128, IT * H], F8, isOutput=False)
    c0_in = nc.declare_dram_parameter("c0_init", [128, IT * H], F32, isOutput=False)
    c1_in = nc.declare_dram_parameter("c1_init", [128, IT * H], F32, isOutput=False)
    mask_in = nc.declare_dram_parameter("mask", [128, 4], F32, isOutput=False)
    out_ext = nc.declare_dram_parameter("out", [2, L, 128, IT * H], F32, isOutput=True)

    MULT = mybir.AluOpType.mult
    ADD = mybir.AluOpType.add
    SIG = mybir.ActivationFunctionType.Sigmoid
    TANH = mybir.ActivationFunctionType.Tanh

    with tile.TileContext(nc) as tc:
        with tc.tile_pool(name="const", bufs=1) as cpool, \
             tc.tile_pool(name="stat", bufs=2) as spool, \
             tc.tile_pool(name="work", bufs=1) as wpool, \
             tc.tile_pool(name="psum", bufs=1, space="PSUM") as ppool, \
             tc.tile_pool(name="dram", bufs=1, space="DRAM") as dpool:

            # ---- phase-1 DMAs: everything conv0_0 + gates0_0 need ----
            gxt_sb = cpool.tile([XR, HALF], BF16, name="gxt_sb", tag="gxt_sb")
            nc.sync.dma_start(gxt_sb[:], gxt_in[:])
            w0h_sb = cpool.tile([H, T * K * DOUT], BF16, name="w0h_sb", tag="w0h_sb")
            nc.sync.dma_start(w0h_sb[:], w0h_in[:])
            w0xb_sb = cpool.tile([XR, T * DOUT], BF16, name="w0xb_sb", tag="w0xb_sb")
            nc.sync.dma_start(w0xb_sb[:], w0xb_in[:])
            # supT1 holds einsum1 output; rows 0:64 preloaded with G@h0_init
            supT1 = [wpool.tile([128, HALF], BF16, name=f"supT1_{k}", tag=f"supT1_{k}")
                     for k in range(K)]
            for k in range(K):
                nc.sync.dma_start(supT1[k][0:H, :], supi_in[:, k * HALF:(k + 1) * HALF])
            c_all = []
            for l, cin in ((0, c0_in), (1, c1_in)):
                ct = cpool.tile([128, IT * H], F32, name=f"c{l}_all", tag=f"c{l}_all")
                nc.sync.dma_start(ct[:], cin[:])
                c_all.append(ct)
            mask_sb = cpool.tile([128, 4], F32, name="mask_sb", tag="mask_sb")
            nc.sync.dma_start(mask_sb[:], mask_in[:])
            mk = [mask_sb[:, 0:1], mask_sb[:, 1:2]]          # 1/0 masks
            mk16 = [mask_sb[:, 2:3], mask_sb[:, 3:4]]        # 16/0 masks
            sc16 = cpool.tile([128, 1], F32, name="sc16", tag="sc16")
            nc.vector.memset(sc16[:], HS)
            h1i_sb = cpool.tile([128, IT * H], F8, name="h1i_sb", tag="h1i_sb")
            nc.sync.dma_start(h1i_sb[:], h1i_in[:])

            # dram bounce/output buffers for the pairwise masked ReduceScatter
            # bounce[i][slot s][l] = own h_l * HS * (slot s is partner)
            bounce = [dpool.tile([2, 2, 128, IT * H], F8, name=f"bounce{i}",
                                 tag=f"bounce{i}") for i in range(2)]
            rs_out = [dpool.tile([2, 128, IT * H], F8, name=f"rso{i}", tag=f"rso{i}")
                      for i in range(2)]

            def rs_issue(tb):
                nc.gpsimd.collective_compute(
                    "ReduceScatter", mybir.AluOpType.add, replica_groups=PAIRS,
                    ins=[bounce[tb].opt()], outs=[rs_out[tb].opt()],
                )

            # t=0: send masked init-h1 (already x16 fp8) into bounce slot l=1
            for sslot in range(2):
                hm = wpool.tile([128, IT * H], F8, name="hm", tag=f"hmi{sslot}")
                nc.vector.tensor_scalar_mul(hm[:], h1i_sb[:], mk[sslot])
                nc.sync.dma_start(bounce[0][sslot, 1], hm[:])

            # ---- bulk DMAs ----
            gt_sb = []
            for jp in range(JP):
                t_ = cpool.tile([128, 2 * K * HALF], F8, name=f"gt{jp}", tag=f"gt{jp}")
                nc.sync.dma_start(t_[:], gt_in[jp])
                gt_sb.append(t_)
                if jp == 1:
                    w1_sb = cpool.tile([SC1, T * K * DOUT], BF16, name="w1_sb",
                                       tag="w1_sb")
                    nc.sync.dma_start(w1_sb[:], w1_in[:])
                if jp == 2:
                    b1_sb = cpool.tile([128, T * DOUT], F32, name="b1_sb", tag="b1_sb")
                    nc.sync.dma_start(b1_sb[:], b1_in[:])

            def conv_mms(pc, t, it, l):
                """One [128, DOUT] psum accumulation for node tile it."""
                if l == 0:
                    for k in range(K):
                        nc.tensor.matmul(
                            pc[:],
                            supT1[k][0:H, it * 128:(it + 1) * 128],
                            w0h_sb[:, (t * K + k) * DOUT:(t * K + k + 1) * DOUT],
                            start=(k == 0), stop=False)
                    nc.tensor.matmul(
                        pc[:],
                        gxt_sb[:, it * 128:(it + 1) * 128],
                        w0xb_sb[:, t * DOUT:(t + 1) * DOUT],
                        start=False, stop=True)
                else:
                    for k in range(K):
                        nc.tensor.matmul(
                            pc[:],
                            supT1[k][:, it * 128:(it + 1) * 128],
                            w1_sb[:, (t * K + k) * DOUT:(t * K + k + 1) * DOUT],
                            start=(k == 0), stop=(k == K - 1))

            # staging for masked sends: one [128, 512] tile per (slot, layer);
            # both ih halves written, then a single contiguous DMA per tile
            hm_cache = {}

            def masked_half_send(l, tb, ih, sig_o, tanh_c):
                """stage sig_o * mask16_s * tanh_c; DMA full tile on 2nd half."""
                for sslot in range(2):
                    if ih == 0:
                        hm_cache[l, sslot] = wpool.tile(
                            [128, IT * H], F8, name="hm", tag=f"hm{sslot}_{l}",
                            bufs=2)
                    hm = hm_cache[l, sslot]
                    nc.vector.scalar_tensor_tensor(
                        hm[:, ih * 256:(ih + 1) * 256], sig_o[:], mk16[sslot],
                        tanh_c[:], MULT, MULT)
                    if ih == 1:
                        nc.sync.dma_start(bounce[tb][sslot, l], hm[:])

            def einsum2_and_gates(t, l, conv_all, c_t, h_dst_fn, send_fn,
                                  h_scaled=True, h2_dst_fn=None):
                """Per half ih: conv psum -> sbuf, LSTM gates, h writes/sends."""
                for ih in range(2):
                    for it in range(ih * 4, ih * 4 + 4):
                        pc = ppool.tile([128, DOUT], F32, name="e2p", tag="e2p", bufs=2)
                        conv_mms(pc, t, it, l)
                        dst = conv_all[:, it * DOUT:(it + 1) * DOUT]
                        if l == 0:
                            if it % 2 == 0:
                                nc.vector.tensor_copy(dst, pc[:])
                            else:
                                nc.scalar.copy(dst, pc[:])
                        else:
                            nc.vector.tensor_tensor(
                                dst, pc[:], b1_sb[:, t * DOUT:(t + 1) * DOUT], ADD)
                    # gates on this half: [128, 4*64] batched ops
                    HB = 4 * H
                    cv = conv_all[:, ih * 4 * DOUT:(ih + 1) * 4 * DOUT].rearrange(
                        "p (it g c) -> p it g c", g=4, c=H)
                    sig_i = wpool.tile([128, HB], F32, name="g_si", tag="g_si", bufs=2)
                    sig_f = wpool.tile([128, HB], F32, name="g_sf", tag="g_sf", bufs=2)
                    sig_o = wpool.tile([128, HB], F32, name="g_so", tag="g_so", bufs=2)
                    tanh_g = wpool.tile([128, HB], F32, name="g_tg", tag="g_tg", bufs=2)
                    nc.scalar.activation(sig_f[:], cv[:, :, 1, :], SIG)
                    nc.scalar.activation(sig_i[:], cv[:, :, 0, :], SIG)
                    nc.scalar.activation(tanh_g[:], cv[:, :, 3, :], TANH)
                    nc.scalar.activation(sig_o[:], cv[:, :, 2, :], SIG)
                    m1 = wpool.tile([128, HB], F32, name="g_m1", tag="g_m1", bufs=2)
                    m2 = wpool.tile([128, HB], F32, name="g_m2", tag="g_m2", bufs=2)
                    ch = c_t[:, ih * HB:(ih + 1) * HB]
                    nc.vector.tensor_tensor(m1[:], sig_f[:], ch, MULT)
                    nc.vector.tensor_tensor(m2[:], sig_i[:], tanh_g[:], MULT)
                    nc.vector.tensor_tensor(ch, m1[:], m2[:], ADD)
                    tanh_c = wpool.tile([128, HB], F32, name="g_tc", tag="g_tc", bufs=2)
                    nc.scalar.activation(tanh_c[:], ch, TANH)
                    if h_scaled:
                        nc.vector.scalar_tensor_tensor(
                            h_dst_fn(ih), sig_o[:], sc16[:, 0:1], tanh_c[:],
                            MULT, MULT)
                    else:
                        nc.vector.tensor_tensor(
                            h_dst_fn(ih), sig_o[:], tanh_c[:], MULT)
                    if h2_dst_fn is not None:
                        nc.vector.tensor_tensor(
                            h2_dst_fn(ih), sig_o[:], tanh_c[:], MULT)
                    if send_fn is not None:
                        send_fn(ih, sig_o, tanh_c)

            hf0 = wpool.tile([128, IT * H], F32, name="hf0", tag="hf0")
            hf0v = hf0[:].rearrange("p (it c) -> p it c", c=H)
            hf1 = wpool.tile([128, IT * H], F32, name="hf1", tag="hf1")

            def conv0_block(t, s1v, c0):
                """conv0_t + gates0_t -> h0 (fp8 x16) into stat-own h0 cols."""
                conv0 = wpool.tile([128, IT * DOUT], F32, name="conv0", tag="conv0")
                einsum2_and_gates(
                    t, 0, conv0, c0,
                    lambda ih: s1v[:, ih * 4:(ih + 1) * 4, 0:H],
                    lambda ih, so, tc_: masked_half_send(0, t % 2, ih, so, tc_),
                    h2_dst_fn=(lambda ih: hf0v[:, ih * 4:(ih + 1) * 4, :])
                    if t == T - 1 else None)

            # ---- preamble: conv0_0 + gates0_0 + stat-own_0 assembly ----
            st_own = spool.tile([128, 8 * SC1], F8, name="st_own", tag="st_own")
            sov = st_own[:].rearrange("p (jt c) -> p jt c", c=SC1)
            nc.vector.tensor_copy(
                sov[:, :, H:SC1], h1i_sb[:].rearrange("p (it c) -> p it c", c=H))
            conv0_block(0, sov, c_all[0])
            rs_issue(0)

            for t in range(T):
                # ------------- einsum1_t: supT1 = G-contract of [h0_t|h1_{t-1}]
                e1p = [[ppool.tile([128, 512], F32, name=f"e1p{k}{ih}",
                                   tag=f"e1p{k}{ih}") for ih in range(2)]
                       for k in range(K)]

                def e1_jp(stat, plo, phi):
                    for jp in range(plo, phi):
                        lhs = stat[:, (jp - plo) * 2 * SC1:
                                   (jp - plo + 1) * 2 * SC1].rearrange(
                            "p (ko c) -> p ko c", ko=2)
                        gv = gt_sb[jp][:].rearrange(
                            "p (ko k i) -> p ko k i", ko=2, k=K)
                        for k in range(K):
                            for ih in range(2):
                                # two accumulation groups (own / partner) so the
                                # own-half MMs don't inherit the partner-load dep
                                nc.tensor.matmul(
                                    e1p[k][ih][:],
                                    lhs,
                                    gv[:, :, k, ih * 512:(ih + 1) * 512],
                                    start=(jp == 0), stop=(jp == phi - 1),
                                    perf_mode=DR, skip_group_check=(plo > 0))

                e1_jp(st_own, 0, 4)

                # partner halves arrive at static offsets: plain DMAs from rs_out
                st_par = spool.tile([128, 8 * SC1], F8, name="st_par", tag="st_par")
                spv = st_par[:].rearrange("p (jt c) -> p jt c", c=SC1)
                nc.sync.dma_start(
                    spv[:, :, 0:H],
                    rs_out[t % 2][0].rearrange("p (it c) -> p it c", c=H))
                nc.scalar.dma_start(
                    spv[:, :, H:SC1],
                    rs_out[t % 2][1].rearrange("p (it c) -> p it c", c=H))
                e1_jp(st_par, 4, 8)

                # evacuate psum -> supT1 (bf16), alternating engines
                for k in range(K):
                    for ih in range(2):
                        dst = supT1[k][:, ih * 512:(ih + 1) * 512]
                        if (k + ih) % 2 == 1:
                            nc.scalar.copy(dst, e1p[k][ih][:])
                        else:
                            nc.vector.tensor_copy(dst, e1p[k][ih][:])

                # ------------- conv1_t + gates1_t first (RS-critical input)
                prev_sov = sov
                if t + 1 < T:
                    st_own = spool.tile([128, 8 * SC1], F8, name="st_own",
                                        tag="st_own")
                    sov = st_own[:].rearrange("p (jt c) -> p jt c", c=SC1)
                conv1 = wpool.tile([128, IT * DOUT], F32, name="conv1", tag="conv1")
                if t + 1 < T:
                    sv = sov
                    h1_dst = lambda ih: sv[:, ih * 4:(ih + 1) * 4, H:SC1]
                    h1_send = lambda ih, so, tc_: masked_half_send(
                        1, (t + 1) % 2, ih, so, tc_)
                    h1_scaled = True
                else:
                    hfv = hf1[:].rearrange("p (it c) -> p it c", c=H)
                    h1_dst = lambda ih: hfv[:, ih * 4:(ih + 1) * 4, :]
                    h1_send = None
                    h1_scaled = False
                einsum2_and_gates(t, 1, conv1, c_all[1], h1_dst, h1_send,
                                  h_scaled=h1_scaled)

                # ------------- conv0_{t+1} + gates0_{t+1}, then the step RS
                if t + 1 < T:
                    conv0_block(t + 1, sov, c_all[0])
                    rs_issue((t + 1) % 2)

            # ---------------- outputs ----------------
            nc.sync.dma_start(out_ext[0, 0], hf0[:])
            nc.sync.dma_start(out_ext[0, 1], hf1[:])
            nc.sync.dma_start(out_ext[1, 0], c_all[0][:])
            nc.sync.dma_start(out_ext[1, 1], c_all[1][:])

    nc.compile()
    _CACHE["nc"] = nc
    return nc


def _host_prep(inputs):
    """Per-core input maps."""
    G = np.asarray(inputs["G"], np.float32)
    x_seq = np.asarray(inputs["x_seq"], np.float32)
    init_h = np.asarray(inputs["init_h"], np.float32)
    init_c = np.asarray(inputs["init_c"], np.float32)
    x_meta = np.asarray(inputs["x_meta"], np.float32)

    def mlp(b, w1, b1, w2, b2):
        hid = np.maximum(x_meta[b] @ w1 + b1, 0.0)
        return hid @ w2 + b2

    # GX[b, k, t, c, i] = sum_j G[k, i, j] x_seq[b, t, j, c]   (full-N once)
    xf = x_seq.transpose(2, 0, 1, 3).reshape(N, B * T * C)
    gx = (G.reshape(K * N, N) @ xf).reshape(K, N, B, T, C)

    WS = GS * HS  # 8192: scale divided out of the h-side conv weights

    in_maps = []
    for c in range(NCORES):
        b, half = c // 2, c % 2
        own = np.arange(half * HALF, (half + 1) * HALF)
        par = np.arange((1 - half) * HALF, (2 - half) * HALF)
        jperm = np.concatenate([own, par])

        # GT[k, j_local, i_own], paired j-tiles for DoubleRow:
        # gt2[jp, p, ko, k, i] = GT[k, (2jp+ko)*128 + p, i] * GS
        gt = G[:, own, :].transpose(0, 2, 1)[:, jperm, :].reshape(K, JT, 128, HALF)
        gt2 = gt.transpose(1, 2, 0, 3).reshape(JP, 2, 128, K, HALF)
        gt2 = gt2.transpose(0, 2, 1, 3, 4).reshape(JP, 128, 2 * K * HALF) * GS
        gt2 = np.ascontiguousarray(gt2).astype(E4)

        # x/bias stationary: rows k*16 + 2t + c = GX[k,t,c,own]; row 48 = 1
        gxt = np.ones((XR, HALF), np.float32)
        gxt[:XR - 1] = gx[:, own, b].transpose(0, 2, 3, 1).reshape(XR - 1, HALF)

        # layer-0 weights
        W0 = mlp(b, inputs["lw1_0"], inputs["lb1_0"], inputs["lw2_0"], inputs["lb2_0"])
        W0 = np.asarray(W0, np.float32).reshape(T, K, DIN0, DOUT)
        bias0 = np.asarray(
            mlp(b, inputs["bw1_0"], inputs["bb1_0"], inputs["bw2_0"], inputs["bb2_0"]),
            np.float32)
        # h-part: rows = h feature (64), cols = (t, k, DOUT); undo fp8 scales
        w0h = W0[:, :, C:, :].transpose(2, 0, 1, 3).reshape(H, T * K * DOUT) / WS
        # x/bias part: [49, T*DOUT]; rows k*16+2t+c nonzero only in col-block t
        w0xb = np.zeros((XR, T, DOUT), np.float32)
        for t in range(T):
            w0xb[np.arange(K)[:, None] * (2 * T) + 2 * t + np.arange(2)[None, :],
                 t] = W0[t, :, :C, :]
        w0xb[XR - 1] = bias0.reshape(T, DOUT)
        w0xb = w0xb.reshape(XR, T * DOUT)

        # layer-1 weights
        W1 = mlp(b, inputs["lw1_1"], inputs["lb1_1"], inputs["lw2_1"], inputs["lb2_1"])
        W1 = np.asarray(W1, np.float32).reshape(T, K, DIN1, DOUT)
        w1 = W1.transpose(2, 0, 1, 3).reshape(SC1, T * K * DOUT) / WS
        bias1 = np.asarray(
            mlp(b, inputs["bw1_1"], inputs["bb1_1"], inputs["bw2_1"], inputs["bb2_1"]),
            np.float32)
        b1 = np.ascontiguousarray(
            np.broadcast_to(bias1.reshape(1, T * DOUT), (128, T * DOUT)))

        # supT1 rows 0:64 preload = (G[k][own] @ h0_init).T * WS  [H, K*HALF]
        h0i = init_h[0, b]
        if np.any(h0i):
            supi = np.stack([(G[k][own] @ h0i).T for k in range(K)], 0) * WS
        else:
            supi = np.zeros((K, H, HALF), np.float32)
        supi = supi.transpose(1, 0, 2).reshape(H, K * HALF)

        h1i = init_h[1, b][own].reshape(IT, 128, H).transpose(1, 0, 2).reshape(
            128, IT * H) * HS
        c0 = init_c[0, b][own].reshape(IT, 128, H).transpose(1, 0, 2).reshape(
            128, IT * H)
        c1 = init_c[1, b][own].reshape(IT, 128, H).transpose(1, 0, 2).reshape(
            128, IT * H)

        msk = np.array([1 - half, half, HS * (1 - half), HS * half], np.float32)

        in_maps.append({
            "gt2": gt2,
            "gxt": np.ascontiguousarray(gxt).astype(BF),
            "w0h": np.ascontiguousarray(w0h).astype(BF),
            "w0xb": np.ascontiguousarray(w0xb).astype(BF),
            "w1": np.ascontiguousarray(w1).astype(BF),
            "bias1": b1,
            "sup_init": np.ascontiguousarray(supi).astype(BF),
            "h1_init": np.ascontiguousarray(h1i).astype(E4),
            "c0_init": np.ascontiguousarray(c0, np.float32),
            "c1_init": np.ascontiguousarray(c1, np.float32),
            "mask": np.ascontiguousarray(np.broadcast_to(
                msk.reshape(1, 4), (128, 4))),
        })
    return in_maps


def kernel(**inputs) -> np.ndarray:
    global LAST_RESULT
    nc = _build()
    in_maps = _host_prep(inputs)
    res = run_bass_kernel_spmd(nc, in_maps, list(range(NCORES)))
    LAST_RESULT = res

    out = np.zeros((2, L, B, N, H), np.float32)
    for c in range(NCORES):
        b, half = c // 2, c % 2
        o = res.results[c]["out"].reshape(2, L, 128, IT, H)
        # node = half*1024 + it*128 + p
        out[:, :, b, half * HALF:(half + 1) * HALF, :] = o.transpose(
            0, 1, 3, 2, 4).reshape(2, L, HALF, H)
    return out


# revision 19
# speedup vs baseline: 1.0517x; 1.0517x over previous
"""Trainium2 Bass kernel for nn_Encoder_61177514164477 (meta-GCN LSTM encoder).

Sharding: 8 cores = 4 batch groups x 2 node-halves. Core c handles batch
b = c//2 and node rows [half*1024, (half+1)*1024) with half = c%2.
G^T (fp8 e4m3, x512) stays SBUF-resident per core; one pairwise masked
ReduceScatter per timestep exchanges h0/h1 (fp8, x16) between the two
halves of each batch pair.

Layer-0's einsum1 (G @ [x_t | h0]) is eliminated: G@h0_t already falls out
of layer-1's einsum1 (rows 0:64 of supT1 = G @ [h0_t | h1_{t-1}]), so
layer-0's conv at step t+1 reuses supT1 rows 0:64 as its stationary. The
tiny known-ahead G@x_t part is computed on the host and shipped as a
49-row bf16 stationary (48 GX rows for all (k,t,c) plus a ones row that
carries the conv bias through the weight matrix).

einsum1 runs in fp8 DoubleRow mode (2 j-tiles per MM, 2x PE throughput):
G scaled x512 and h x16 to stay in e4m3 range; the 8192x factor is divided
out of the host-computed W0h/W1. Offline sim: end-to-end rel err ~1.0e-2.

Per step the PE does:
  einsum1: 8 jp x 6 (k,ih) DoubleRow MMs @N=512 (single-phase PSUM accum)
  conv0:   8 it x (3 h-MMs contraction 64 + 1 x/bias-MM contraction 49) @N=256
  conv1:   8 it x 3 k MMs @N=256   (bf16)
"""
import os
import numpy as np
import ml_dtypes

import concourse.bass as bass
import concourse.mybir as mybir
import concourse.tile as tile
import concourse.bacc as bacc
import concourse.tile_utils as tile_utils
from concourse.bass_utils import run_bass_kernel_spmd

# use the full cayman SBUF (224 KiB phys / ~208 usable per partition)
tile_utils.max_sbuf_usage = 204 * 1024

L, B, T, N, C, H, K, M = 2, 4, 8, 2048, 2, 64, 3, 32
DIN0, DIN1, DOUT = C + H, 2 * H, 4 * H
HALF = N // 2          # 1024 rows per core
JT = N // 128          # 16 j-tiles (local order: 8 own + 8 partner)
JP = JT // 2           # 8 j-tile pairs (DoubleRow: 4 own + 4 partner)
IT = HALF // 128       # 8 own i-tiles
NCORES = 8
PAIRS = [[0, 1], [2, 3], [4, 5], [6, 7]]

F32 = mybir.dt.float32
BF16 = mybir.dt.bfloat16
F8 = mybir.dt.float8e4
DR = mybir.MatmulPerfMode.DoubleRow
BF = ml_dtypes.bfloat16
E4 = ml_dtypes.float8_e4m3fn

SC1 = DIN1             # 128 stationary cols per j-tile ([h0|h1])
XR = K * T * C + 1     # 49 rows of the x/bias stationary (48 GX + ones)
GS = 512.0             # G fp8 scale
HS = 16.0              # h fp8 scale

_CACHE = {}
LAST_RESULT = None


def _build():
    if "nc" in _CACHE:
        return _CACHE["nc"]
    nc = bacc.Bacc(None, target_bir_lowering=False, debug=False)

    gt_in = nc.declare_dram_parameter("gt2", [JP, 128, 2 * K * HALF], F8,
                                      isOutput=False)
    gxt_in = nc.declare_dram_parameter("gxt", [XR, HALF], BF16, isOutput=False)
    w0h_in = nc.declare_dram_parameter("w0h", [H, T * K * DOUT], BF16, isOutput=False)
    w0xb_in = nc.declare_dram_parameter("w0xb", [XR, T * DOUT], BF16, isOutput=False)
    w1_in = nc.declare_dram_parameter("w1", [SC1, T * K * DOUT], BF16, isOutput=False)
    b1_in = nc.declare_dram_parameter("bias1", [128, T * DOUT], F32, isOutput=False)
    supi_in = nc.declare_dram_parameter("sup_init", [H, K * HALF], BF16, isOutput=False)
    h1i_in = nc.declare_dram_parameter("h1_init", [128, IT * H], F8, isOutput=False)
    c0_in = nc.declare_dram_parameter("c0_init", [128, IT * H], F32, isOutput=False)
    c1_in = nc.declare_dram_parameter("c1_init", [128, IT * H], F32, isOutput=False)
    mask_in = nc.declare_dram_parameter("mask", [128, 4], F32, isOutput=False)
    out_ext = nc.declare_dram_parameter("out", [2, L, 128, IT * H], F32, isOutput=True)

    MULT = mybir.AluOpType.mult
    ADD = mybir.AluOpType.add
    SIG = mybir.ActivationFunctionType.Sigmoid
    TANH = mybir.ActivationFunctionType.Tanh

    with tile.TileContext(nc) as tc:
        with tc.tile_pool(name="const", bufs=1) as cpool, \
             tc.tile_pool(name="stat", bufs=2) as spool, \
             tc.tile_pool(name="work", bufs=1) as wpool, \
             tc.tile_pool(name="psum", bufs=1, space="PSUM") as ppool, \
             tc.tile_pool(name="dram", bufs=1, space="DRAM") as dpool:

            # ---- phase-1 DMAs: everything conv0_0 + gates0_0 need ----
            gxt_sb = cpool.tile([XR, HALF], BF16, name="gxt_sb", tag="gxt_sb")
            nc.sync.dma_start(gxt_sb[:], gxt_in[:])
            w0h_sb = cpool.tile([H, T * K * DOUT], BF16, name="w0h_sb", tag="w0h_sb")
            nc.sync.dma_start(w0h_sb[:], w0h_in[:])
            w0xb_sb = cpool.tile([XR, T * DOUT], BF16, name="w0xb_sb", tag="w0xb_sb")
            nc.sync.dma_start(w0xb_sb[:], w0xb_in[:])
            # supT1 holds einsum1 output; rows 0:64 preloaded with G@h0_init
            supT1 = [wpool.tile([128, HALF], BF16, name=f"supT1_{k}", tag=f"supT1_{k}")
                     for k in range(K)]
            for k in range(K):
                nc.sync.dma_start(supT1[k][0:H, :], supi_in[:, k * HALF:(k + 1) * HALF])
            c_all = []
            for l, cin in ((0, c0_in), (1, c1_in)):
                ct = cpool.tile([128, IT * H], F32, name=f"c{l}_all", tag=f"c{l}_all")
                nc.sync.dma_start(ct[:], cin[:])
                c_all.append(ct)
            mask_sb = cpool.tile([128, 4], F32, name="mask_sb", tag="mask_sb")
            nc.sync.dma_start(mask_sb[:], mask_in[:])
            mk = [mask_sb[:, 0:1], mask_sb[:, 1:2]]          # 1/0 masks
            mk16 = [mask_sb[:, 2:3], mask_sb[:, 3:4]]        # 16/0 masks
            sc16 = cpool.tile([128, 1], F32, name="sc16", tag="sc16")
            nc.vector.memset(sc16[:], HS)
            h1i_sb = cpool.tile([128, IT * H], F8, name="h1i_sb", tag="h1i_sb")
            nc.sync.dma_start(h1i_sb[:], h1i_in[:])

            # dram bounce/output buffers for the pairwise masked ReduceScatter
            # bounce[i][slot s][l] = own h_l * HS * (slot s is partner)
            bounce = [dpool.tile([2, 2, 128, IT * H], F8, name=f"bounce{i}",
                                 tag=f"bounce{i}") for i in range(2)]
            rs_out = [dpool.tile([2, 128, IT * H], F8, name=f"rso{i}", tag=f"rso{i}")
                      for i in range(2)]

            def rs_issue(tb):
                nc.gpsimd.collective_compute(
                    "ReduceScatter", mybir.AluOpType.add, replica_groups=PAIRS,
                    ins=[bounce[tb].opt()], outs=[rs_out[tb].opt()],
                )

            # tiny dummy collective issued first: absorbs CC-core boot (~60us)
            # while the bulk DMAs run, so the first real RS processes promptly
            dum_in = dpool.tile([2, 128, 16], BF16, name="dum_in", tag="dum_in")
            dum_out = dpool.tile([128, 16], BF16, name="dum_out", tag="dum_out")
            dum_sb = wpool.tile([128, 16], BF16, name="dum_sb", tag="dum_sb")
            nc.vector.memset(dum_sb[:], 0.0)
            nc.sync.dma_start(dum_in[0], dum_sb[:])
            nc.sync.dma_start(dum_in[1], dum_sb[:])
            nc.gpsimd.collective_compute(
                "ReduceScatter", mybir.AluOpType.add, replica_groups=PAIRS,
                ins=[dum_in.opt()], outs=[dum_out.opt()],
            )

            # t=0: send masked init-h1 (already x16 fp8) into bounce slot l=1
            for sslot in range(2):
                hm = wpool.tile([128, IT * H], F8, name="hm", tag=f"hmi{sslot}")
                nc.vector.tensor_scalar_mul(hm[:], h1i_sb[:], mk[sslot])
                nc.sync.dma_start(bounce[0][sslot, 1], hm[:])

            # ---- bulk DMAs ----
            gt_sb = []
            for jp in range(JP):
                t_ = cpool.tile([128, 2 * K * HALF], F8, name=f"gt{jp}", tag=f"gt{jp}")
                nc.sync.dma_start(t_[:], gt_in[jp])
                gt_sb.append(t_)
                if jp == 1:
                    w1_sb = cpool.tile([SC1, T * K * DOUT], BF16, name="w1_sb",
                                       tag="w1_sb")
                    nc.sync.dma_start(w1_sb[:], w1_in[:])
                if jp == 2:
                    b1_sb = cpool.tile([128, T * DOUT], F32, name="b1_sb", tag="b1_sb")
                    nc.sync.dma_start(b1_sb[:], b1_in[:])

            def conv_mms(pc, t, it, l):
                """One [128, DOUT] psum accumulation for node tile it."""
                if l == 0:
                    for k in range(K):
                        nc.tensor.matmul(
                            pc[:],
                            supT1[k][0:H, it * 128:(it + 1) * 128],
                            w0h_sb[:, (t * K + k) * DOUT:(t * K + k + 1) * DOUT],
                            start=(k == 0), stop=False)
                    nc.tensor.matmul(
                        pc[:],
                        gxt_sb[:, it * 128:(it + 1) * 128],
                        w0xb_sb[:, t * DOUT:(t + 1) * DOUT],
                        start=False, stop=True)
                else:
                    for k in range(K):
                        nc.tensor.matmul(
                            pc[:],
                            supT1[k][:, it * 128:(it + 1) * 128],
                            w1_sb[:, (t * K + k) * DOUT:(t * K + k + 1) * DOUT],
                            start=(k == 0), stop=(k == K - 1))

            # staging for masked sends: one [128, 512] tile per (slot, layer);
            # both ih halves written, then a single contiguous DMA per tile
            hm_cache = {}

            def masked_half_send(l, tb, ih, sig_o, tanh_c):
                """stage sig_o * mask16_s * tanh_c; DMA full tile on 2nd half."""
                for sslot in range(2):
                    if ih == 0:
                        hm_cache[l, sslot] = wpool.tile(
                            [128, IT * H], F8, name="hm", tag=f"hm{sslot}_{l}",
                            bufs=2)
                    hm = hm_cache[l, sslot]
                    nc.vector.scalar_tensor_tensor(
                        hm[:, ih * 256:(ih + 1) * 256], sig_o[:], mk16[sslot],
                        tanh_c[:], MULT, MULT)
                    if ih == 1:
                        nc.sync.dma_start(bounce[tb][sslot, l], hm[:])

            def einsum2_and_gates(t, l, conv_all, c_t, h_dst_fn, send_fn,
                                  h_scaled=True, h2_dst_fn=None):
                """Per half ih: conv psum -> sbuf, LSTM gates, h writes/sends."""
                for ih in range(2):
                    for it in range(ih * 4, ih * 4 + 4):
                        pc = ppool.tile([128, DOUT], F32, name="e2p", tag="e2p", bufs=2)
                        conv_mms(pc, t, it, l)
                        dst = conv_all[:, it * DOUT:(it + 1) * DOUT]
                        if l == 0:
                            if it % 2 == 0:
                                nc.vector.tensor_copy(dst, pc[:])
                            else:
                                nc.scalar.copy(dst, pc[:])
                        else:
                            nc.vector.tensor_tensor(
                                dst, pc[:], b1_sb[:, t * DOUT:(t + 1) * DOUT], ADD)
                    # gates on this half: [128, 4*64] batched ops
                    HB = 4 * H
                    cv = conv_all[:, ih * 4 * DOUT:(ih + 1) * 4 * DOUT].rearrange(
                        "p (it g c) -> p it g c", g=4, c=H)
                    sig_i = wpool.tile([128, HB], F32, name="g_si", tag="g_si", bufs=2)
                    sig_f = wpool.tile([128, HB], F32, name="g_sf", tag="g_sf", bufs=2)
                    sig_o = wpool.tile([128, HB], F32, name="g_so", tag="g_so", bufs=2)
                    tanh_g = wpool.tile([128, HB], F32, name="g_tg", tag="g_tg", bufs=2)
                    nc.scalar.activation(sig_f[:], cv[:, :, 1, :], SIG)
                    nc.scalar.activation(sig_i[:], cv[:, :, 0, :], SIG)
                    nc.scalar.activation(tanh_g[:], cv[:, :, 3, :], TANH)
                    nc.scalar.activation(sig_o[:], cv[:, :, 2, :], SIG)
                    m1 = wpool.tile([128, HB], F32, name="g_m1", tag="g_m1", bufs=2)
                    m2 = wpool.tile([128, HB], F32, name="g_m2", tag="g_m2", bufs=2)
                    ch = c_t[:, ih * HB:(ih + 1) * HB]
                    nc.vector.tensor_tensor(m1[:], sig_f[:], ch, MULT)
                    nc.vector.tensor_tensor(m2[:], sig_i[:], tanh_g[:], MULT)
                    nc.vector.tensor_tensor(ch, m1[:], m2[:], ADD)
                    tanh_c = wpool.tile([128, HB], F32, name="g_tc", tag="g_tc", bufs=2)
                    nc.scalar.activation(tanh_c[:], ch, TANH)
                    if h_scaled:
                        nc.vector.scalar_tensor_tensor(
                            h_dst_fn(ih), sig_o[:], sc16[:, 0:1], tanh_c[:],
                            MULT, MULT)
                    else:
                        nc.vector.tensor_tensor(
                            h_dst_fn(ih), sig_o[:], tanh_c[:], MULT)
                    if h2_dst_fn is not None:
                        nc.vector.tensor_tensor(
                            h2_dst_fn(ih), sig_o[:], tanh_c[:], MULT)
                    if send_fn is not None:
                        send_fn(ih, sig_o, tanh_c)

            hf0 = wpool.tile([128, IT * H], F32, name="hf0", tag="hf0")
            hf0v = hf0[:].rearrange("p (it c) -> p it c", c=H)
            hf1 = wpool.tile([128, IT * H], F32, name="hf1", tag="hf1")

            def conv0_block(t, s1v, c0):
                """conv0_t + gates0_t -> h0 (fp8 x16) into stat-own h0 cols."""
                conv0 = wpool.tile([128, IT * DOUT], F32, name="conv0", tag="conv0")
                einsum2_and_gates(
                    t, 0, conv0, c0,
                    lambda ih: s1v[:, ih * 4:(ih + 1) * 4, 0:H],
                    lambda ih, so, tc_: masked_half_send(0, t % 2, ih, so, tc_),
                    h2_dst_fn=(lambda ih: hf0v[:, ih * 4:(ih + 1) * 4, :])
                    if t == T - 1 else None)

            # ---- preamble: conv0_0 + gates0_0 + stat-own_0 assembly ----
            st_own = spool.tile([128, 8 * SC1], F8, name="st_own", tag="st_own")
            sov = st_own[:].rearrange("p (jt c) -> p jt c", c=SC1)
            nc.vector.tensor_copy(
                sov[:, :, H:SC1], h1i_sb[:].rearrange("p (it c) -> p it c", c=H))
            conv0_block(0, sov, c_all[0])
            rs_issue(0)

            for t in range(T):
                # ------------- einsum1_t: supT1 = G-contract of [h0_t|h1_{t-1}]
                e1p = [[ppool.tile([128, 512], F32, name=f"e1p{k}{ih}",
                                   tag=f"e1p{k}{ih}") for ih in range(2)]
                       for k in range(K)]

                def e1_jp(stat, plo, phi, ih):
                    for jp in range(plo, phi):
                        lhs = stat[:, (jp - plo) * 2 * SC1:
                                   (jp - plo + 1) * 2 * SC1].rearrange(
                            "p (ko c) -> p ko c", ko=2)
                        gv = gt_sb[jp][:].rearrange(
                            "p (ko k i) -> p ko k i", ko=2, k=K)
                        for k in range(K):
                            # two accumulation groups (own / partner) so the
                            # own-half MMs don't inherit the partner-load dep
                            nc.tensor.matmul(
                                e1p[k][ih][:],
                                lhs,
                                gv[:, :, k, ih * 512:(ih + 1) * 512],
                                start=(jp == 0), stop=(jp == phi - 1),
                                perf_mode=DR, skip_group_check=(plo > 0))

                def e1_evac(ih):
                    # psum -> supT1 (bf16), alternating engines
                    for k in range(K):
                        dst = supT1[k][:, ih * 512:(ih + 1) * 512]
                        if (k + ih) % 2 == 1:
                            nc.scalar.copy(dst, e1p[k][ih][:])
                        else:
                            nc.vector.tensor_copy(dst, e1p[k][ih][:])

                e1_jp(st_own, 0, 4, 0)
                e1_jp(st_own, 0, 4, 1)

                # partner halves arrive at static offsets: plain DMAs from rs_out
                st_par = spool.tile([128, 8 * SC1], F8, name="st_par", tag="st_par")
                spv = st_par[:].rearrange("p (jt c) -> p jt c", c=SC1)
                nc.sync.dma_start(
                    spv[:, :, 0:H],
                    rs_out[t % 2][0].rearrange("p (it c) -> p it c", c=H))
                nc.scalar.dma_start(
                    spv[:, :, H:SC1],
                    rs_out[t % 2][1].rearrange("p (it c) -> p it c", c=H))
                # ih-split: evac + conv/gates of the first i-half overlap the
                # PE running the second i-half's partner MMs
                e1_jp(st_par, 4, 8, 0)
                e1_evac(0)
                e1_jp(st_par, 4, 8, 1)
                e1_evac(1)

                # ------------- conv1_t + gates1_t first (RS-critical input)
                prev_sov = sov
                if t + 1 < T:
                    st_own = spool.tile([128, 8 * SC1], F8, name="st_own",
                                        tag="st_own")
                    sov = st_own[:].rearrange("p (jt c) -> p jt c", c=SC1)
                conv1 = wpool.tile([128, IT * DOUT], F32, name="conv1", tag="conv1")
                if t + 1 < T:
                    sv = sov
                    h1_dst = lambda ih: sv[:, ih * 4:(ih + 1) * 4, H:SC1]
                    h1_send = lambda ih, so, tc_: masked_half_send(
                        1, (t + 1) % 2, ih, so, tc_)
                    h1_scaled = True
                else:
                    hfv = hf1[:].rearrange("p (it c) -> p it c", c=H)
                    h1_dst = lambda ih: hfv[:, ih * 4:(ih + 1) * 4, :]
                    h1_send = None
                    h1_scaled = False
                einsum2_and_gates(t, 1, conv1, c_all[1], h1_dst, h1_send,
                                  h_scaled=h1_scaled)

                # ------------- conv0_{t+1} + gates0_{t+1}, then the step RS
                if t + 1 < T:
                    conv0_block(t + 1, sov, c_all[0])
                    rs_issue((t + 1) % 2)

            # ---------------- outputs ----------------
            nc.sync.dma_start(out_ext[0, 0], hf0[:])
            nc.sync.dma_start(out_ext[0, 1], hf1[:])
            nc.sync.dma_start(out_ext[1, 0], c_all[0][:])
            nc.sync.dma_start(out_ext[1, 1], c_all[1][:])

    nc.compile()
    _CACHE["nc"] = nc
    return nc


def _host_prep(inputs):
    """Per-core input maps."""
    G = np.asarray(inputs["G"], np.float32)
    x_seq = np.asarray(inputs["x_seq"], np.float32)
    init_h = np.asarray(inputs["init_h"], np.float32)
    init_c = np.asarray(inputs["init_c"], np.float32)
    x_meta = np.asarray(inputs["x_meta"], np.float32)

    def mlp(b, w1, b1, w2, b2):
        hid = np.maximum(x_meta[b] @ w1 + b1, 0.0)
        return hid @ w2 + b2

    # GX[b, k, t, c, i] = sum_j G[k, i, j] x_seq[b, t, j, c]   (full-N once)
    xf = x_seq.transpose(2, 0, 1, 3).reshape(N, B * T * C)
    gx = (G.reshape(K * N, N) @ xf).reshape(K, N, B, T, C)

    WS = GS * HS  # 8192: scale divided out of the h-side conv weights

    in_maps = []
    for c in range(NCORES):
        b, half = c // 2, c % 2
        own = np.arange(half * HALF, (half + 1) * HALF)
        par = np.arange((1 - half) * HALF, (2 - half) * HALF)
        jperm = np.concatenate([own, par])

        # GT[k, j_local, i_own], paired j-tiles for DoubleRow:
        # gt2[jp, p, ko, k, i] = GT[k, (2jp+ko)*128 + p, i] * GS
        gt = G[:, own, :].transpose(0, 2, 1)[:, jperm, :].reshape(K, JT, 128, HALF)
        gt2 = gt.transpose(1, 2, 0, 3).reshape(JP, 2, 128, K, HALF)
        gt2 = gt2.transpose(0, 2, 1, 3, 4).reshape(JP, 128, 2 * K * HALF) * GS
        gt2 = np.ascontiguousarray(gt2).astype(E4)

        # x/bias stationary: rows k*16 + 2t + c = GX[k,t,c,own]; row 48 = 1
        gxt = np.ones((XR, HALF), np.float32)
        gxt[:XR - 1] = gx[:, own, b].transpose(0, 2, 3, 1).reshape(XR - 1, HALF)

        # layer-0 weights
        W0 = mlp(b, inputs["lw1_0"], inputs["lb1_0"], inputs["lw2_0"], inputs["lb2_0"])
        W0 = np.asarray(W0, np.float32).reshape(T, K, DIN0, DOUT)
        bias0 = np.asarray(
            mlp(b, inputs["bw1_0"], inputs["bb1_0"], inputs["bw2_0"], inputs["bb2_0"]),
            np.float32)
        # h-part: rows = h feature (64), cols = (t, k, DOUT); undo fp8 scales
        w0h = W0[:, :, C:, :].transpose(2, 0, 1, 3).reshape(H, T * K * DOUT) / WS
        # x/bias part: [49, T*DOUT]; rows k*16+2t+c nonzero only in col-block t
        w0xb = np.zeros((XR, T, DOUT), np.float32)
        for t in range(T):
            w0xb[np.arange(K)[:, None] * (2 * T) + 2 * t + np.arange(2)[None, :],
                 t] = W0[t, :, :C, :]
        w0xb[XR - 1] = bias0.reshape(T, DOUT)
        w0xb = w0xb.reshape(XR, T * DOUT)

        # layer-1 weights
        W1 = mlp(b, inputs["lw1_1"], inputs["lb1_1"], inputs["lw2_1"], inputs["lb2_1"])
        W1 = np.asarray(W1, np.float32).reshape(T, K, DIN1, DOUT)
        w1 = W1.transpose(2, 0, 1, 3).reshape(SC1, T * K * DOUT) / WS
        bias1 = np.asarray(
            mlp(b, inputs["bw1_1"], inputs["bb1_1"], inputs["bw2_1"], inputs["bb2_1"]),
            np.float32)
        b1 = np.ascontiguousarray(
            np.broadcast_to(bias1.reshape(1, T * DOUT), (128, T * DOUT)))

        # supT1 rows 0:64 preload = (G[k][own] @ h0_init).T * WS  [H, K*HALF]
        h0i = init_h[0, b]
        if np.any(h0i):
            supi = np.stack([(G[k][own] @ h0i).T for k in range(K)], 0) * WS
        else:
            supi = np.zeros((K, H, HALF), np.float32)
        supi = supi.transpose(1, 0, 2).reshape(H, K * HALF)

        h1i = init_h[1, b][own].reshape(IT, 128, H).transpose(1, 0, 2).reshape(
            128, IT * H) * HS
        c0 = init_c[0, b][own].reshape(IT, 128, H).transpose(1, 0, 2).reshape(
            128, IT * H)
        c1 = init_c[1, b][own].reshape(IT, 128, H).transpose(1, 0, 2).reshape(
            128, IT * H)

        msk = np.array([1 - half, half, HS * (1 - half), HS * half], np.float32)

        in_maps.append({
            "gt2": gt2,
            "gxt": np.ascontiguousarray(gxt).astype(BF),
            "w0h": np.ascontiguousarray(w0h).astype(BF),
            "w0xb": np.ascontiguousarray(w0xb).astype(BF),
            "w1": np.ascontiguousarray(w1).astype(BF),
            "bias1": b1,
            "sup_init": np.ascontiguousarray(supi).astype(BF),
            "h1_init": np.ascontiguousarray(h1i).astype(E4),
            "c0_init": np.ascontiguousarray(c0, np.float32),
            "c1_init": np.ascontiguousarray(c1, np.float32),
            "mask": np.ascontiguousarray(np.broadcast_to(
                msk.reshape(1, 4), (128, 4))),
        })
    return in_maps


def kernel(**inputs) -> np.ndarray:
    global LAST_RESULT
    nc = _build()
    in_maps = _host_prep(inputs)
    res = run_bass_kernel_spmd(nc, in_maps, list(range(NCORES)))
    LAST_RESULT = res

    out = np.zeros((2, L, B, N, H), np.float32)
    for c in range(NCORES):
        b, half = c // 2, c % 2
        o = res.results[c]["out"].reshape(2, L, 128, IT, H)
        # node = half*1024 + it*128 + p
        out[:, :, b, half * HALF:(half + 1) * HALF, :] = o.transpose(
            0, 1, 3, 2, 4).reshape(2, L, HALF, H)
    return out


# revision 20
# speedup vs baseline: 1.0603x; 1.0081x over previous
"""Trainium2 Bass kernel for nn_Encoder_61177514164477 (meta-GCN LSTM encoder).

Sharding: 8 cores = 4 batch groups x 2 node-halves. Core c handles batch
b = c//2 and node rows [half*1024, (half+1)*1024) with half = c%2.
G^T (fp8 e4m3, x512) stays SBUF-resident per core; one pairwise masked
ReduceScatter per timestep exchanges h0/h1 (fp8, x16) between the two
halves of each batch pair.

Layer-0's einsum1 (G @ [x_t | h0]) is eliminated: G@h0_t already falls out
of layer-1's einsum1 (rows 0:64 of supT1 = G @ [h0_t | h1_{t-1}]), so
layer-0's conv at step t+1 reuses supT1 rows 0:64 as its stationary. The
tiny known-ahead G@x_t part is computed on the host and shipped as a
49-row bf16 stationary (48 GX rows for all (k,t,c) plus a ones row that
carries the conv bias through the weight matrix).

einsum1 runs in fp8 DoubleRow mode (2 j-tiles per MM, 2x PE throughput):
G scaled x512 and h x16 to stay in e4m3 range; the 8192x factor is divided
out of the host-computed W0h/W1. Offline sim: end-to-end rel err ~1.0e-2.

Per step the PE does:
  einsum1: 8 jp x 6 (k,ih) DoubleRow MMs @N=512 (single-phase PSUM accum)
  conv0:   8 it x (3 h-MMs contraction 64 + 1 x/bias-MM contraction 49) @N=256
  conv1:   8 it x 3 k MMs @N=256   (bf16)
"""
import os
import numpy as np
import ml_dtypes

import concourse.bass as bass
import concourse.mybir as mybir
import concourse.tile as tile
import concourse.bacc as bacc
import concourse.tile_utils as tile_utils
from concourse.bass_utils import run_bass_kernel_spmd

# use the full cayman SBUF (224 KiB phys / ~208 usable per partition)
tile_utils.max_sbuf_usage = 204 * 1024

L, B, T, N, C, H, K, M = 2, 4, 8, 2048, 2, 64, 3, 32
DIN0, DIN1, DOUT = C + H, 2 * H, 4 * H
HALF = N // 2          # 1024 rows per core
JT = N // 128          # 16 j-tiles (local order: 8 own + 8 partner)
JP = JT // 2           # 8 j-tile pairs (DoubleRow: 4 own + 4 partner)
IT = HALF // 128       # 8 own i-tiles
NCORES = 8
PAIRS = [[0, 1], [2, 3], [4, 5], [6, 7]]

F32 = mybir.dt.float32
BF16 = mybir.dt.bfloat16
F8 = mybir.dt.float8e4
DR = mybir.MatmulPerfMode.DoubleRow
BF = ml_dtypes.bfloat16
E4 = ml_dtypes.float8_e4m3fn

SC1 = DIN1             # 128 stationary cols per j-tile ([h0|h1])
XR = K * T * C + 1     # 49 rows of the x/bias stationary (48 GX + ones)
GS = 512.0             # G fp8 scale
HS = 16.0              # h fp8 scale

_CACHE = {}
LAST_RESULT = None


def _build():
    if "nc" in _CACHE:
        return _CACHE["nc"]
    nc = bacc.Bacc(None, target_bir_lowering=False, debug=False)

    gt_in = nc.declare_dram_parameter("gt2", [JP, 128, 2 * K * HALF], F8,
                                      isOutput=False)
    gxt_in = nc.declare_dram_parameter("gxt", [XR, HALF], BF16, isOutput=False)
    w0h_in = nc.declare_dram_parameter("w0h", [H, T * K * DOUT], BF16, isOutput=False)
    w0xb_in = nc.declare_dram_parameter("w0xb", [XR, T * DOUT], BF16, isOutput=False)
    w1_in = nc.declare_dram_parameter("w1", [SC1, T * K * DOUT], BF16, isOutput=False)
    b1_in = nc.declare_dram_parameter("bias1", [128, T * DOUT], F32, isOutput=False)
    supi_in = nc.declare_dram_parameter("sup_init", [H, K * HALF], BF16, isOutput=False)
    h1i_in = nc.declare_dram_parameter("h1_init", [128, IT * H], F8, isOutput=False)
    c0_in = nc.declare_dram_parameter("c0_init", [128, IT * H], F32, isOutput=False)
    c1_in = nc.declare_dram_parameter("c1_init", [128, IT * H], F32, isOutput=False)
    mask_in = nc.declare_dram_parameter("mask", [128, 4], F32, isOutput=False)
    out_ext = nc.declare_dram_parameter("out", [2, L, 128, IT * H], F32, isOutput=True)

    MULT = mybir.AluOpType.mult
    ADD = mybir.AluOpType.add
    SIG = mybir.ActivationFunctionType.Sigmoid
    TANH = mybir.ActivationFunctionType.Tanh

    with tile.TileContext(nc) as tc:
        with tc.tile_pool(name="const", bufs=1) as cpool, \
             tc.tile_pool(name="stat", bufs=2) as spool, \
             tc.tile_pool(name="work", bufs=1) as wpool, \
             tc.tile_pool(name="psum", bufs=1, space="PSUM") as ppool, \
             tc.tile_pool(name="dram", bufs=1, space="DRAM") as dpool:

            # ---- phase-1 DMAs: everything conv0_0 + gates0_0 need ----
            gxt_sb = cpool.tile([XR, HALF], BF16, name="gxt_sb", tag="gxt_sb")
            nc.sync.dma_start(gxt_sb[:], gxt_in[:])
            w0h_sb = cpool.tile([H, T * K * DOUT], BF16, name="w0h_sb", tag="w0h_sb")
            nc.sync.dma_start(w0h_sb[:], w0h_in[:])
            w0xb_sb = cpool.tile([XR, T * DOUT], BF16, name="w0xb_sb", tag="w0xb_sb")
            nc.sync.dma_start(w0xb_sb[:], w0xb_in[:])
            # supT1 holds einsum1 output; rows 0:64 preloaded with G@h0_init
            supT1 = [wpool.tile([128, HALF], BF16, name=f"supT1_{k}", tag=f"supT1_{k}")
                     for k in range(K)]
            for k in range(K):
                nc.sync.dma_start(supT1[k][0:H, :], supi_in[:, k * HALF:(k + 1) * HALF])
            c_all = []
            for l, cin in ((0, c0_in), (1, c1_in)):
                ct = cpool.tile([128, IT * H], F32, name=f"c{l}_all", tag=f"c{l}_all")
                nc.sync.dma_start(ct[:], cin[:])
                c_all.append(ct)
            mask_sb = cpool.tile([128, 4], F32, name="mask_sb", tag="mask_sb")
            nc.sync.dma_start(mask_sb[:], mask_in[:])
            mk = [mask_sb[:, 0:1], mask_sb[:, 1:2]]          # 1/0 masks
            mk16 = [mask_sb[:, 2:3], mask_sb[:, 3:4]]        # 16/0 masks
            sc16 = cpool.tile([128, 1], F32, name="sc16", tag="sc16")
            nc.vector.memset(sc16[:], HS)
            h1i_sb = cpool.tile([128, IT * H], F8, name="h1i_sb", tag="h1i_sb")
            nc.sync.dma_start(h1i_sb[:], h1i_in[:])

            # dram bounce/output buffers for the pairwise masked ReduceScatter
            # bounce[i][slot s][l] = own h_l * HS * (slot s is partner)
            bounce = [dpool.tile([2, 2, 128, IT * H], F8, name=f"bounce{i}",
                                 tag=f"bounce{i}") for i in range(2)]
            rs_out = [dpool.tile([2, 128, IT * H], F8, name=f"rso{i}", tag=f"rso{i}")
                      for i in range(2)]

            def rs_issue(tb):
                nc.gpsimd.collective_compute(
                    "ReduceScatter", mybir.AluOpType.add, replica_groups=PAIRS,
                    ins=[bounce[tb].opt()], outs=[rs_out[tb].opt()],
                )

            # tiny dummy collective issued first: absorbs CC-core boot (~60us)
            # while the bulk DMAs run, so the first real RS processes promptly
            dum_in = dpool.tile([2, 128, 16], BF16, name="dum_in", tag="dum_in")
            dum_out = dpool.tile([128, 16], BF16, name="dum_out", tag="dum_out")
            dum_sb = wpool.tile([128, 16], BF16, name="dum_sb", tag="dum_sb")
            nc.vector.memset(dum_sb[:], 0.0)
            nc.sync.dma_start(dum_in[0], dum_sb[:])
            nc.sync.dma_start(dum_in[1], dum_sb[:])
            nc.gpsimd.collective_compute(
                "ReduceScatter", mybir.AluOpType.add, replica_groups=PAIRS,
                ins=[dum_in.opt()], outs=[dum_out.opt()],
            )

            # t=0: send masked init-h1 (already x16 fp8) into bounce slot l=1
            for sslot in range(2):
                hm = wpool.tile([128, IT * H], F8, name="hm", tag=f"hmi{sslot}")
                nc.vector.tensor_scalar_mul(hm[:], h1i_sb[:], mk[sslot])
                nc.sync.dma_start(bounce[0][sslot, 1], hm[:])

            # ---- bulk DMAs ----
            gt_sb = []
            for jp in range(JP):
                t_ = cpool.tile([128, 2 * K * HALF], F8, name=f"gt{jp}", tag=f"gt{jp}")
                nc.sync.dma_start(t_[:], gt_in[jp])
                gt_sb.append(t_)
                if jp == 1:
                    w1_sb = cpool.tile([SC1, T * K * DOUT], BF16, name="w1_sb",
                                       tag="w1_sb")
                    nc.sync.dma_start(w1_sb[:], w1_in[:])
                if jp == 2:
                    b1_sb = cpool.tile([128, T * DOUT], F32, name="b1_sb", tag="b1_sb")
                    nc.sync.dma_start(b1_sb[:], b1_in[:])

            def conv_mms(pc, t, it, l):
                """One [128, DOUT] psum accumulation for node tile it."""
                if l == 0:
                    for k in range(K):
                        nc.tensor.matmul(
                            pc[:],
                            supT1[k][0:H, it * 128:(it + 1) * 128],
                            w0h_sb[:, (t * K + k) * DOUT:(t * K + k + 1) * DOUT],
                            start=(k == 0), stop=False)
                    nc.tensor.matmul(
                        pc[:],
                        gxt_sb[:, it * 128:(it + 1) * 128],
                        w0xb_sb[:, t * DOUT:(t + 1) * DOUT],
                        start=False, stop=True)
                else:
                    for k in range(K):
                        nc.tensor.matmul(
                            pc[:],
                            supT1[k][:, it * 128:(it + 1) * 128],
                            w1_sb[:, (t * K + k) * DOUT:(t * K + k + 1) * DOUT],
                            start=(k == 0), stop=(k == K - 1))

            def masked_half_send(l, tb, ih, sig_o, tanh_c):
                """bounce[tb][s][l][:, ih half] <- sig_o * mask16_s * tanh_c."""
                for sslot in range(2):
                    hm = wpool.tile([128, 4 * H], F8, name="hm", tag=f"hm{sslot}",
                                    bufs=2)
                    nc.vector.scalar_tensor_tensor(
                        hm[:], sig_o[:], mk16[sslot], tanh_c[:], MULT, MULT)
                    nc.sync.dma_start(
                        bounce[tb][sslot, l][:, ih * 256:(ih + 1) * 256], hm[:])

            def einsum2_and_gates(t, l, conv_all, c_t, h_dst_fn, send_fn,
                                  h_scaled=True, h2_dst_fn=None):
                """Per half ih: conv psum -> sbuf, LSTM gates, h writes/sends."""
                for ih in range(2):
                    for it in range(ih * 4, ih * 4 + 4):
                        pc = ppool.tile([128, DOUT], F32, name="e2p", tag="e2p", bufs=2)
                        conv_mms(pc, t, it, l)
                        dst = conv_all[:, it * DOUT:(it + 1) * DOUT]
                        if l == 0:
                            if it % 2 == 0:
                                nc.vector.tensor_copy(dst, pc[:])
                            else:
                                nc.scalar.copy(dst, pc[:])
                        else:
                            nc.vector.tensor_tensor(
                                dst, pc[:], b1_sb[:, t * DOUT:(t + 1) * DOUT], ADD)
                    # gates on this half: [128, 4*64] batched ops
                    HB = 4 * H
                    cv = conv_all[:, ih * 4 * DOUT:(ih + 1) * 4 * DOUT].rearrange(
                        "p (it g c) -> p it g c", g=4, c=H)
                    sig_i = wpool.tile([128, HB], F32, name="g_si", tag="g_si", bufs=2)
                    sig_f = wpool.tile([128, HB], F32, name="g_sf", tag="g_sf", bufs=2)
                    sig_o = wpool.tile([128, HB], F32, name="g_so", tag="g_so", bufs=2)
                    tanh_g = wpool.tile([128, HB], F32, name="g_tg", tag="g_tg", bufs=2)
                    nc.scalar.activation(sig_f[:], cv[:, :, 1, :], SIG)
                    nc.scalar.activation(sig_i[:], cv[:, :, 0, :], SIG)
                    nc.scalar.activation(tanh_g[:], cv[:, :, 3, :], TANH)
                    nc.scalar.activation(sig_o[:], cv[:, :, 2, :], SIG)
                    m1 = wpool.tile([128, HB], F32, name="g_m1", tag="g_m1", bufs=2)
                    m2 = wpool.tile([128, HB], F32, name="g_m2", tag="g_m2", bufs=2)
                    ch = c_t[:, ih * HB:(ih + 1) * HB]
                    nc.vector.tensor_tensor(m1[:], sig_f[:], ch, MULT)
                    nc.vector.tensor_tensor(m2[:], sig_i[:], tanh_g[:], MULT)
                    nc.vector.tensor_tensor(ch, m1[:], m2[:], ADD)
                    tanh_c = wpool.tile([128, HB], F32, name="g_tc", tag="g_tc", bufs=2)
                    nc.scalar.activation(tanh_c[:], ch, TANH)
                    if h_scaled:
                        nc.vector.scalar_tensor_tensor(
                            h_dst_fn(ih), sig_o[:], sc16[:, 0:1], tanh_c[:],
                            MULT, MULT)
                    else:
                        nc.vector.tensor_tensor(
                            h_dst_fn(ih), sig_o[:], tanh_c[:], MULT)
                    if h2_dst_fn is not None:
                        nc.vector.tensor_tensor(
                            h2_dst_fn(ih), sig_o[:], tanh_c[:], MULT)
                    if send_fn is not None:
                        send_fn(ih, sig_o, tanh_c)

            hf0 = wpool.tile([128, IT * H], F32, name="hf0", tag="hf0")
            hf0v = hf0[:].rearrange("p (it c) -> p it c", c=H)
            hf1 = wpool.tile([128, IT * H], F32, name="hf1", tag="hf1")

            def conv0_block(t, s1v, c0):
                """conv0_t + gates0_t -> h0 (fp8 x16) into stat-own h0 cols."""
                conv0 = wpool.tile([128, IT * DOUT], F32, name="conv0", tag="conv0")
                einsum2_and_gates(
                    t, 0, conv0, c0,
                    lambda ih: s1v[:, ih * 4:(ih + 1) * 4, 0:H],
                    lambda ih, so, tc_: masked_half_send(0, t % 2, ih, so, tc_),
                    h2_dst_fn=(lambda ih: hf0v[:, ih * 4:(ih + 1) * 4, :])
                    if t == T - 1 else None)

            # ---- preamble: conv0_0 + gates0_0 + stat-own_0 assembly ----
            st_own = spool.tile([128, 8 * SC1], F8, name="st_own", tag="st_own")
            sov = st_own[:].rearrange("p (jt c) -> p jt c", c=SC1)
            nc.vector.tensor_copy(
                sov[:, :, H:SC1], h1i_sb[:].rearrange("p (it c) -> p it c", c=H))
            conv0_block(0, sov, c_all[0])
            rs_issue(0)

            for t in range(T):
                # ------------- einsum1_t: supT1 = G-contract of [h0_t|h1_{t-1}]
                e1p = [[ppool.tile([128, 512], F32, name=f"e1p{k}{ih}",
                                   tag=f"e1p{k}{ih}") for ih in range(2)]
                       for k in range(K)]

                def e1_jp(stat, plo, phi, ih):
                    for jp in range(plo, phi):
                        lhs = stat[:, (jp - plo) * 2 * SC1:
                                   (jp - plo + 1) * 2 * SC1].rearrange(
                            "p (ko c) -> p ko c", ko=2)
                        gv = gt_sb[jp][:].rearrange(
                            "p (ko k i) -> p ko k i", ko=2, k=K)
                        for k in range(K):
                            # two accumulation groups (own / partner) so the
                            # own-half MMs don't inherit the partner-load dep
                            nc.tensor.matmul(
                                e1p[k][ih][:],
                                lhs,
                                gv[:, :, k, ih * 512:(ih + 1) * 512],
                                start=(jp == 0), stop=(jp == phi - 1),
                                perf_mode=DR, skip_group_check=(plo > 0))

                def e1_evac(ih):
                    # psum -> supT1 (bf16), alternating engines
                    for k in range(K):
                        dst = supT1[k][:, ih * 512:(ih + 1) * 512]
                        if (k + ih) % 2 == 1:
                            nc.scalar.copy(dst, e1p[k][ih][:])
                        else:
                            nc.vector.tensor_copy(dst, e1p[k][ih][:])

                e1_jp(st_own, 0, 4, 0)
                e1_jp(st_own, 0, 4, 1)

                # partner halves arrive at static offsets: plain DMAs from rs_out
                st_par = spool.tile([128, 8 * SC1], F8, name="st_par", tag="st_par")
                spv = st_par[:].rearrange("p (jt c) -> p jt c", c=SC1)
                nc.sync.dma_start(
                    spv[:, :, 0:H],
                    rs_out[t % 2][0].rearrange("p (it c) -> p it c", c=H))
                nc.scalar.dma_start(
                    spv[:, :, H:SC1],
                    rs_out[t % 2][1].rearrange("p (it c) -> p it c", c=H))
                # ih-split: evac + conv/gates of the first i-half overlap the
                # PE running the second i-half's partner MMs
                e1_jp(st_par, 4, 8, 0)
                e1_evac(0)
                e1_jp(st_par, 4, 8, 1)
                e1_evac(1)

                # ------------- conv1_t + gates1_t first (RS-critical input)
                prev_sov = sov
                if t + 1 < T:
                    st_own = spool.tile([128, 8 * SC1], F8, name="st_own",
                                        tag="st_own")
                    sov = st_own[:].rearrange("p (jt c) -> p jt c", c=SC1)
                conv1 = wpool.tile([128, IT * DOUT], F32, name="conv1", tag="conv1")
                if t + 1 < T:
                    sv = sov
                    h1_dst = lambda ih: sv[:, ih * 4:(ih + 1) * 4, H:SC1]
                    h1_send = lambda ih, so, tc_: masked_half_send(
                        1, (t + 1) % 2, ih, so, tc_)
                    h1_scaled = True
                else:
                    hfv = hf1[:].rearrange("p (it c) -> p it c", c=H)
                    h1_dst = lambda ih: hfv[:, ih * 4:(ih + 1) * 4, :]
                    h1_send = None
                    h1_scaled = False
                einsum2_and_gates(t, 1, conv1, c_all[1], h1_dst, h1_send,
                                  h_scaled=h1_scaled)

                # ------------- conv0_{t+1} + gates0_{t+1}, then the step RS
                if t + 1 < T:
                    conv0_block(t + 1, sov, c_all[0])
                    rs_issue((t + 1) % 2)

            # ---------------- outputs ----------------
            nc.sync.dma_start(out_ext[0, 0], hf0[:])
            nc.sync.dma_start(out_ext[0, 1], hf1[:])
            nc.sync.dma_start(out_ext[1, 0], c_all[0][:])
            nc.sync.dma_start(out_ext[1, 1], c_all[1][:])

    nc.compile()
    _CACHE["nc"] = nc
    return nc


def _host_prep(inputs):
    """Per-core input maps."""
    G = np.asarray(inputs["G"], np.float32)
    x_seq = np.asarray(inputs["x_seq"], np.float32)
    init_h = np.asarray(inputs["init_h"], np.float32)
    init_c = np.asarray(inputs["init_c"], np.float32)
    x_meta = np.asarray(inputs["x_meta"], np.float32)

    def mlp(b, w1, b1, w2, b2):
        hid = np.maximum(x_meta[b] @ w1 + b1, 0.0)
        return hid @ w2 + b2

    # GX[b, k, t, c, i] = sum_j G[k, i, j] x_seq[b, t, j, c]   (full-N once)
    xf = x_seq.transpose(2, 0, 1, 3).reshape(N, B * T * C)
    gx = (G.reshape(K * N, N) @ xf).reshape(K, N, B, T, C)

    WS = GS * HS  # 8192: scale divided out of the h-side conv weights

    in_maps = []
    for c in range(NCORES):
        b, half = c // 2, c % 2
        own = np.arange(half * HALF, (half + 1) * HALF)
        par = np.arange((1 - half) * HALF, (2 - half) * HALF)
        jperm = np.concatenate([own, par])

        # GT[k, j_local, i_own], paired j-tiles for DoubleRow:
        # gt2[jp, p, ko, k, i] = GT[k, (2jp+ko)*128 + p, i] * GS
        gt = G[:, own, :].transpose(0, 2, 1)[:, jperm, :].reshape(K, JT, 128, HALF)
        gt2 = gt.transpose(1, 2, 0, 3).reshape(JP, 2, 128, K, HALF)
        gt2 = gt2.transpose(0, 2, 1, 3, 4).reshape(JP, 128, 2 * K * HALF) * GS
        gt2 = np.ascontiguousarray(gt2).astype(E4)

        # x/bias stationary: rows k*16 + 2t + c = GX[k,t,c,own]; row 48 = 1
        gxt = np.ones((XR, HALF), np.float32)
        gxt[:XR - 1] = gx[:, own, b].transpose(0, 2, 3, 1).reshape(XR - 1, HALF)

        # layer-0 weights
        W0 = mlp(b, inputs["lw1_0"], inputs["lb1_0"], inputs["lw2_0"], inputs["lb2_0"])
        W0 = np.asarray(W0, np.float32).reshape(T, K, DIN0, DOUT)
        bias0 = np.asarray(
            mlp(b, inputs["bw1_0"], inputs["bb1_0"], inputs["bw2_0"], inputs["bb2_0"]),
            np.float32)
        # h-part: rows = h feature (64), cols = (t, k, DOUT); undo fp8 scales
        w0h = W0[:, :, C:, :].transpose(2, 0, 1, 3).reshape(H, T * K * DOUT) / WS
        # x/bias part: [49, T*DOUT]; rows k*16+2t+c nonzero only in col-block t
        w0xb = np.zeros((XR, T, DOUT), np.float32)
        for t in range(T):
            w0xb[np.arange(K)[:, None] * (2 * T) + 2 * t + np.arange(2)[None, :],
                 t] = W0[t, :, :C, :]
        w0xb[XR - 1] = bias0.reshape(T, DOUT)
        w0xb = w0xb.reshape(XR, T * DOUT)

        # layer-1 weights
        W1 = mlp(b, inputs["lw1_1"], inputs["lb1_1"], inputs["lw2_1"], inputs["lb2_1"])
        W1 = np.asarray(W1, np.float32).reshape(T, K, DIN1, DOUT)
        w1 = W1.transpose(2, 0, 1, 3).reshape(SC1, T * K * DOUT) / WS
        bias1 = np.asarray(
            mlp(b, inputs["bw1_1"], inputs["bb1_1"], inputs["bw2_1"], inputs["bb2_1"]),
            np.float32)
        b1 = np.ascontiguousarray(
            np.broadcast_to(bias1.reshape(1, T * DOUT), (128, T * DOUT)))

        # supT1 rows 0:64 preload = (G[k][own] @ h0_init).T * WS  [H, K*HALF]
        h0i = init_h[0, b]
        if np.any(h0i):
            supi = np.stack([(G[k][own] @ h0i).T for k in range(K)], 0) * WS
        else:
            supi = np.zeros((K, H, HALF), np.float32)
        supi = supi.transpose(1, 0, 2).reshape(H, K * HALF)

        h1i = init_h[1, b][own].reshape(IT, 128, H).transpose(1, 0, 2).reshape(
            128, IT * H) * HS
        c0 = init_c[0, b][own].reshape(IT, 128, H).transpose(1, 0, 2).reshape(
            128, IT * H)
        c1 = init_c[1, b][own].reshape(IT, 128, H).transpose(1, 0, 2).reshape(
            128, IT * H)

        msk = np.array([1 - half, half, HS * (1 - half), HS * half], np.float32)

        in_maps.append({
            "gt2": gt2,
            "gxt": np.ascontiguousarray(gxt).astype(BF),
            "w0h": np.ascontiguousarray(w0h).astype(BF),
            "w0xb": np.ascontiguousarray(w0xb).astype(BF),
            "w1": np.ascontiguousarray(w1).astype(BF),
            "bias1": b1,
            "sup_init": np.ascontiguousarray(supi).astype(BF),
            "h1_init": np.ascontiguousarray(h1i).astype(E4),
            "c0_init": np.ascontiguousarray(c0, np.float32),
            "c1_init": np.ascontiguousarray(c1, np.float32),
            "mask": np.ascontiguousarray(np.broadcast_to(
                msk.reshape(1, 4), (128, 4))),
        })
    return in_maps


def kernel(**inputs) -> np.ndarray:
    global LAST_RESULT
    nc = _build()
    in_maps = _host_prep(inputs)
    res = run_bass_kernel_spmd(nc, in_maps, list(range(NCORES)))
    LAST_RESULT = res

    out = np.zeros((2, L, B, N, H), np.float32)
    for c in range(NCORES):
        b, half = c // 2, c % 2
        o = res.results[c]["out"].reshape(2, L, 128, IT, H)
        # node = half*1024 + it*128 + p
        out[:, :, b, half * HALF:(half + 1) * HALF, :] = o.transpose(
            0, 1, 3, 2, 4).reshape(2, L, HALF, H)
    return out
